# revision 19
# baseline (speedup 1.0000x reference)
"""Bradley-Terry loss kernel for Trainium2 — symmetrized Chebyshev/PE design.

Since softplus(d) - softplus(-d) = d, the loss splits into a symmetric
part and a rank-1 correction:

  loss = sum_{i!=j} W_ij sp(b_j - b_i)
       = 1/4 sum_{i!=j} S_ij g(d_ij)  +  1/2 (b . colsum(W) - b . rowsum(W))

with S = W + W^T, d_ij = b_j - b_i, g(d) = sp(d) + sp(-d) (even).  S and
g are symmetric, so each unordered block-pair of a 16x512 blocking needs
streaming only ONCE (doubled afterwards; diagonal blocks pre-scaled by
1/2): HBM traffic drops from 64MB to 34MB.  A circulant tournament
orients pair {a, b} toward column b iff (b-a) mod 16 in 1..7 (ties at 8
go to the lower column), so every column v < 8 receives exactly 9 blocks
and every v >= 8 exactly 8.  Core c owns columns {c, c+8} = 17 blocks =
4.25MB, one uniform SPMD instruction stream (chains of 9 and 8), and
each Y column is written by exactly one core (128KB out per core).

g(h*(y-x)) is approximated by a degree-31 tensor-product Chebyshev
expansion (~4e-7), so each block is a matmul against the Chebyshev basis
of its row range: Y[m, j] += sum_i S_ij T_m(x_i).  S streams in fp8 e4m3
(RNE rounding washes out to ~3e-5 over the sum); the basis is scaled
double-fp8 [Q8(T) | Q8(16(T-Q8(T)))] -> M=64, combined on the host as
Y_hi + Y_lo/16.  All-fp8 operands enable DoubleRow perf mode (row pairs
(i, i+128) interleaved host-side; 256-row contraction per matmul).
PSUM accumulates fp32 down each block-column chain; Y leaves in bf16.
Per-slot basis blocks are baked into each core's inputs so lhsT offsets
stay static.  Dummy matmuls on memset tiles run during the DMA head so
the PE's HAM clock gate is already released when real work arrives.
The O(N) remainder (hi/lo combine, stage-2 with the exact fp64 basis,
A-contraction, row/col-sum corrections) runs in float64 on the host.
"""

import numpy as np
import ml_dtypes

import concourse.bacc as bacc
import concourse.bass as bass
import concourse.mybir as mybir
from concourse import tile
from concourse.bass_utils import run_bass_kernel_spmd

N = 8192
NCORES = 8
P = 128                    # SBUF partitions
BLK = 512                  # block size
NB = N // BLK              # 16 blocks
NDT = BLK // 256           # 2 DoubleRow tiles (256 rows) per block
UNITW = NDT * BLK * 2      # 2048 B per partition per unit: [dt][c][pair]
CROWW = NDT * 2 * 64       # 256 B per partition per unit basis: [dt][ko][m]
CHAINS = (9, 8)            # units per owned column (v < 8, v >= 8)
NUNITS = sum(CHAINS)       # 17
DEG = 31
M1 = DEG + 1               # 32 chebyshev coefficients
M2 = 2 * M1                # hi + lo stacked -> 64 stationary columns
NWARM = 40                 # HAM warm-up matmuls during the DMA head
_LN2 = float(np.log(2.0))

_cached_nc = None


def _col_rows(v):
    """Row-blocks feeding column v under the circulant orientation."""
    rows = [v] + [(v - k) % NB for k in range(1, 8)]
    if v < NB // 2:
        rows.append(v + NB // 2)
    return rows


def _cheb_vals(x, deg):
    out = np.empty((len(x), deg + 1), dtype=np.float64)
    out[:, 0] = 1.0
    if deg >= 1:
        out[:, 1] = x
    for k in range(2, deg + 1):
        out[:, k] = 2 * x * out[:, k - 1] - out[:, k - 2]
    return out


def _cheb2d_coeffs(f, deg):
    n = deg + 1
    theta = (np.arange(n) + 0.5) * np.pi / n
    pts = np.cos(theta)
    F = f(pts[:, None], pts[None, :])
    Tm = np.cos(np.outer(np.arange(n), theta))
    A = (2.0 / n) * Tm @ F @ ((2.0 / n) * Tm).T
    A[0, :] /= 2
    A[:, 0] /= 2
    return A


def _build():
    nc = bacc.Bacc(
        "TRN2",
        target_bir_lowering=False,
        debug=False,
        enable_asserts=False,
        num_devices=NCORES,
    )
    f32 = mybir.dt.float32
    bf16 = mybir.dt.bfloat16
    fp8 = mybir.dt.float8e4

    # DMA-facing tensors are declared f32 over the same bytes: DMA
    # descriptors cap at 4096 ELEMENTS, so fp8-typed transfers split into
    # 4KB packets (~190 GB/s/queue) while f32-typed ones get 8KB (~310).
    s = nc.dram_tensor("s", [P, NUNITS * UNITW // 4], f32, kind="ExternalInput")
    crows = nc.dram_tensor(
        "crows", [P, NUNITS * CROWW // 4], f32, kind="ExternalInput"
    )
    y = nc.dram_tensor("y", [M2, 2 * BLK], bf16, kind="ExternalOutput")

    with tile.TileContext(nc) as tc:
        with (
            tc.tile_pool(name="consts", bufs=1) as consts,
            tc.tile_pool(name="spool", bufs=1) as spool,
            tc.tile_pool(name="ypool", bufs=2) as ypool,
            tc.tile_pool(name="psum", bufs=2, space="PSUM") as pspool,
        ):
            crows_sb = consts.tile([P, NUNITS * CROWW // 4], f32)
            ch_ = CHAINS[0] * CROWW // 4
            nc.sync.dma_start(crows_sb[:, :ch_], crows.ap()[:, :ch_])
            nc.scalar.dma_start(crows_sb[:, ch_:], crows.ap()[:, ch_:])

            # HAM warm-up: short matmuls on memset tiles keep the PE busy
            # through the clock-gate window while the first chunks land.
            warm_c = consts.tile([P, M2], fp8)
            warm_w = consts.tile([P, 128], fp8)
            nc.vector.memset(warm_c[:], 1.0)
            nc.vector.memset(warm_w[:], 1.0)
            wps = pspool.tile([M2, 128], f32, tag="warm", name="warm_ps")
            for k in range(NWARM):
                nc.tensor.matmul(wps[:], warm_c[:], warm_w[:], start=True, stop=True)

            # Units grouped into DMA chunks strictly alternating between the
            # two HWDGE queues in consumption order: each queue's FIFO then
            # delivers in the order the PE consumes, so neither chain stalls
            # behind a later-needed chunk.  4-unit groups give 8KB
            # per-partition runs (full-size packets).
            UW4 = UNITW // 4
            groups = [(0, [(4, nc.scalar), (5, nc.sync)]),
                      (1, [(4, nc.scalar), (4, nc.sync)])]
            tiles = {}
            slot0 = 0
            gi = 0
            for chain, sizes in groups:
                base = slot0
                off = 0
                for sz, eng in sizes:
                    stile = spool.tile([P, sz * UW4], f32, tag=f"s{gi % 3}")
                    gi += 1
                    lo_ = (base + off) * UW4
                    eng.dma_start(stile[:], s.ap()[:, lo_ : lo_ + sz * UW4])
                    for j in range(sz):
                        tiles[base + off + j] = (stile, j)
                    off += sz
                slot0 += off

            slot = 0
            for chain, nu in enumerate(CHAINS):
                ps = pspool.tile([M2, BLK], f32, tag=f"ps{chain}")
                for k in range(nu):
                    st, j = tiles[slot]
                    for dt in range(NDT):
                        co = (slot * CROWW + dt * (CROWW // NDT)) // 4
                        lhsT = (
                            crows_sb[:, co : co + CROWW // NDT // 4]
                            .bitcast(fp8)
                            .rearrange("p (ko m) -> p ko m", ko=2)
                        )
                        ro = (j * UNITW + dt * BLK * 2) // 4
                        rhs = (
                            st[:, ro : ro + BLK * 2 // 4]
                            .bitcast(fp8)
                            .rearrange("p (n two) -> p two n", two=2)
                        )
                        nc.tensor.matmul(
                            ps[:],
                            lhsT,
                            rhs,
                            start=(k == 0 and dt == 0),
                            stop=(k == nu - 1 and dt == NDT - 1),
                            perf_mode=mybir.MatmulPerfMode.DoubleRow,
                        )
                    # fillers: keep the HAM clock gate open across DMA gaps
                    nc.tensor.matmul(wps[:], warm_c[:], warm_w[:], start=True, stop=True)
                    nc.tensor.matmul(wps[:], warm_c[:], warm_w[:], start=True, stop=True)
                    slot += 1
                yh = ypool.tile([M2, BLK], bf16, tag="y")
                nc.vector.tensor_copy(yh[:], ps[:])
                nc.scalar.dma_start(
                    y.ap()[:, chain * BLK : (chain + 1) * BLK], yh[:]
                )

    nc.compile()
    return nc


def _get_nc():
    global _cached_nc
    if _cached_nc is None:
        _cached_nc = _build()
    return _cached_nc


def _q8(x):
    return x.astype(ml_dtypes.float8_e4m3)


def _pack_unit(block8):
    """[512, 512] fp8 -> [128, 2048] per-partition [dt][c][pair] layout."""
    return np.ascontiguousarray(
        block8.reshape(NDT, 2, P, BLK).transpose(2, 0, 3, 1).reshape(P, UNITW)
    )


def _pack_crows(c2blk):
    """[512, 64] fp8 basis rows -> [128, 256] per-partition [dt][ko][m]."""
    return np.ascontiguousarray(
        c2blk.reshape(NDT, 2, P, M2).transpose(2, 0, 1, 3).reshape(P, CROWW)
    )


def kernel(win_matrix, betas, _trace=False):
    win_matrix = np.asarray(win_matrix, dtype=np.float32)
    betas = np.asarray(betas, dtype=np.float32)
    nc = _get_nc()

    b64 = betas.astype(np.float64)
    lo, hi = float(b64.min()), float(b64.max())
    c = 0.5 * (lo + hi)
    h = max(0.5 * (hi - lo) * 1.000001, 1e-12)
    x = (b64 - c) / h

    def g(X, Y):
        d = h * (Y - X)
        return np.logaddexp(0.0, d) + np.logaddexp(0.0, -d)

    Ag = _cheb2d_coeffs(g, DEG)
    C = _cheb_vals(x, DEG)                       # [N, 32] f64
    C_hi = _q8(C)
    C_lo = _q8(16.0 * (C - C_hi.astype(np.float64)))
    C2 = np.concatenate([C_hi, C_lo], axis=1)    # [N, 64] fp8

    S = win_matrix + win_matrix.T                # [N, N] f32
    dvals = np.diagonal(win_matrix).astype(np.float64)
    colsum = win_matrix.sum(axis=0, dtype=np.float64)
    rowsum = win_matrix.sum(axis=1, dtype=np.float64)
    corr = 0.5 * (b64 @ colsum - b64 @ rowsum)
    dq = float(_q8(dvals.astype(np.float32)).astype(np.float64).sum())

    in_maps = []
    for cc in range(NCORES):
        cols = (cc, cc + NB // 2)
        sbufs, cbufs = [], []
        for v in cols:
            for bi in _col_rows(v):
                blk = S[bi * BLK : (bi + 1) * BLK, v * BLK : (v + 1) * BLK]
                if bi == v:
                    blk = blk * 0.5
                sbufs.append(_pack_unit(_q8(blk)))
                cbufs.append(_pack_crows(C2[bi * BLK : (bi + 1) * BLK]))
        in_maps.append(
            {
                "s": np.concatenate(sbufs, axis=1).view(np.float32),
                "crows": np.concatenate(cbufs, axis=1).view(np.float32),
            }
        )
    res = run_bass_kernel_spmd(
        nc, in_maps, core_ids=list(range(NCORES)), trace=_trace
    )

    Yfull = np.zeros((M2, N), dtype=np.float64)
    for cc in range(NCORES):
        yv = res.results[cc]["y"].astype(np.float64)
        for chain, v in enumerate((cc, cc + NB // 2)):
            Yfull[:, v * BLK : (v + 1) * BLK] = yv[:, chain * BLK : (chain + 1) * BLK]
    Yc = Yfull[:M1] + Yfull[M1:] / 16.0
    z = Yc @ C                                    # [32, 32]
    D = float((Ag * z).sum())
    total = 0.5 * D - _LN2 * dq + corr
    if _trace:
        kernel.last_results = res
    return np.array(total, dtype=np.float32)
